# revision 15
# baseline (speedup 1.0000x reference)
"""Trainium2 Bass kernel for nn_Enhancement_11819749999257.

Computes: 3x (1x1-conv MLP w/ BN+relu) feature embeddings + soft scatter of
per-joint features onto a 7x7 grid ("bone projection"), concatenated.

Full output: (256, 4736, 7, 7) f32 = 237 MB  -> memory(write)-bound.
Per core: 29.7 MB of stores at the ~360 GB/s HBM wall ~= 82 us; the craft
is all in the ramp (first store ASAP), keeping the SDMA engines gap-free,
and the tail (every engine finishing together).

Strategy (pure data parallel over batch, 8 cores x 32 batch items):
  - n = b_local*74 + j  flattens (batch item, joint). Per-core output
    (32, 4736, 49) is contiguous as rows n: out[n, c*49+s]. Rows are
    processed in 19 slots of 128 partitions. NLOC = 2368 = 19*128 - 64;
    the 64 pad rows are placed where they help the DMA tail:
      * 52 pads on partitions 0-51 of slot 9 (mid-stream),
      * 12 pads on SDMA-engine-15's partitions {124-127 of slot 17,
        92-95 & 124-127 of slot 18}: engine 15 drains ~6% slower than
        the others (known TRN2 trait), so skipping its descriptors in
        the final slots lets all 16 engines finish together.
  - Each slot is ONE store (128 x 12544 B descriptors) issued as soon as
    the slot's DVE multiply finishes; slots 0-2 are split into 4/2/2
    column pieces so the first store issues early. DVE production
    (~3.5us/slot) outruns DMA consumption (~4.2us/slot), so after slot 0
    the two HWDGE rings stream back-to-back stores to the end.
  - MLP: w1/w2 are 64x64; BN (eval) folded into per-channel scale/bias on
    host. PE matmuls: y1 = relu(scale*(w1 @ x) + bias) in column pieces;
    per slot F = [y1;1].T @ [w2.T; b2] (K=65 folds the b2 add) lands in
    PSUM in [n-partition, c-free] layout. The first 128 x-columns ride
    inside the cpa constants DMA so the first matmul waits on a single
    load; the ones row of y1e comes from a tiny DMA (input `onesr`).
    GpSimd (slow Q7 start) is never used.
  - Grid weights W[n, s] = relu(1 - sqrt((gy_s+eps-u_n)^2 + (gx_s+eps-v_n)^2))
    via ACT Square (per-partition bias = -uv), one DVE add per group,
    ACT Sqrt, ACT Relu. The compiler's two ACT table loads (set 0 at
    block entry + the sqrt set) run 6.6-9.2us, overlapping the input
    DMAs; cpb lands ~9.1us so the W chain is not table-bound.
  - Scatter: OUT[n, c*49+s] = F[n, c] * W[n, s] -- one DVE tensor_tensor
    mult per slot with stride-0 broadcast APs.
  - Stores alternate between the two HWDGE rings (scalar / sync); all
    input DMAs except cpb ride sync so the ACT sequencer stays free.
"""

import numpy as np

import concourse.bass as bass
import concourse.mybir as mybir
from concourse import bacc, bass_utils
from concourse.tile import TileContext

F32 = mybir.dt.float32
AF = mybir.ActivationFunctionType
ALU = mybir.AluOpType

N_CORES = 8
B = 256
B_LOC = B // N_CORES      # 32
J = 74                    # 21 + 21 + 32 joints, concat order r, l, o
C = 64
S = 7
S2 = S * S                # 49
NLOC = B_LOC * J          # 2368 rows per core
P = 128
NCHUNK = 19               # slots
NPAD = NCHUNK * P         # 2432
OUT_COLS = C * S2         # 3136
EPS = 1.0e-6
NA = 512                  # max phase-A column piece
# phase-A pieces (col0, width, slots); piece 0 is one slot and rides the
# cpa DMA so the first matmul starts as soon as possible
APIECES = [
    (0, 128, [0]),
    (128, 256, [1, 2]),
    (384, 512, [3, 4, 5, 6]),
    (896, 512, [7, 8, 9, 10]),
    (1408, 512, [11, 12, 13, 14]),
    (1920, 512, [15, 16, 17, 18]),
]
# W-chain batching groups == the APIECES slot lists
# store channel-piece widths per slot: slot 0 finely split for an early
# first store; slots 1-6 split 2x so early stores chase the DVE
# multiplies piece-wise (DVE production only outruns DMA consumption by
# ~15%, so half-slot granularity avoids whole-slot wait bubbles until
# DVE builds a lead)
PIECES = {0: [8, 16, 16, 24], 1: [32, 32], 2: [32, 32], 3: [32, 32],
          4: [32, 32], 5: [32, 32], 6: [32, 32]}

# pad partitions per slot (64 total): 48 mid-stream on slot 9 (its
# 80-partition store sprays all 16 SDMA engines: stores assign
# descriptors to engines in 4-partition slices from engine 0, wrapping
# to all 16 only when the count is divisible by 16), 16 in the last
# slot so its two stores (60 and 52 partitions -> engines 0-14 and
# 0-12) skip exactly the ~7%-slow engine 15, letting its queue drain
# in time with the others' final slot.
PADS = {9: set(range(0, 48)), 18: set(range(112, 128))}
# contiguous valid-partition runs per slot
RUNS = {9: [(48, 128)], 18: [(0, 60), (60, 112)]}

# packed-constants column layout:
# cpax = [w1t|w2b|sc1|bi1|x0] (MLP path, sync ring), cpb = [gyc|gxc|nuv]
# (grid path, scalar ring); onesr = the K=65 ones row (sync ring).
OFF_W1 = 0
OFF_W2B = OFF_W1 + C            # 64
OFF_SC = OFF_W2B + C            # 128
OFF_BI = OFF_SC + 1             # 129
OFF_X0 = OFF_BI + 1             # 130
NCONST_A = OFF_X0 + P           # 258
OFF_GY = 0
OFF_GX = OFF_GY + S2            # 49
OFF_NUV = OFF_GX + S2           # 98
NCONST_B = OFF_NUV + 2 * NCHUNK  # 136


def _slot_layout():
    """Per slot: (dram row0, [(p_lo, p_hi, row0_of_run), ...])."""
    out = []
    r = 0
    for s in range(NCHUNK):
        runs = []
        for (p_lo, p_hi) in RUNS.get(s, [(0, P)]):
            runs.append((p_lo, p_hi, r))
            r += p_hi - p_lo
        out.append(runs)
    assert r == NLOC
    return out


SLOT_RUNS = _slot_layout()


def _build_module():
    nc = bacc.Bacc(None)
    names = {}
    with TileContext(nc) as tc:
        with tc.tile_pool(name="dram", bufs=1, space="DRAM") as dram:
            xall = dram.tile((C, NPAD), F32, kind="ExternalInput", name="xall")
            cpax = dram.tile((P, NCONST_A), F32, kind="ExternalInput", name="cpax")
            cpb = dram.tile((P, NCONST_B), F32, kind="ExternalInput", name="cpb")
            onesr = dram.tile((1, NPAD), F32, kind="ExternalInput", name="onesr")
            out = dram.tile((NLOC, OUT_COLS), F32, kind="ExternalOutput", name="out")
            for key, ap in (("xall", xall), ("cpax", cpax), ("cpb", cpb),
                            ("onesr", onesr), ("out", out)):
                names[key] = ap.tensor.name

            with (
                tc.tile_pool(name="consts", bufs=1) as cpool,
                tc.tile_pool(name="ps_a", bufs=2, space="PSUM") as ps_a,
                tc.tile_pool(name="ps_b", bufs=4, space="PSUM") as ps_b,
                tc.tile_pool(name="outs", bufs=6) as opool,
            ):
                # Warm the ACT sqrt-set table (also holds Square/Relu): the
                # compiler inserts the set-3 table load right before the
                # first Sqrt in ACT program order, so a dependency-light
                # dummy Sqrt up front hoists the ~1.3us load to t~6.6-9.2,
                # overlapping the input DMAs instead of the W chain.
                scr = cpool.tile((1, 8), F32)
                scro = cpool.tile((1, 8), F32)
                nc.vector.memset(scr[:], 0.0625)
                nc.scalar.activation(scro[:, 0:2], scr[:, 0:2], AF.Sqrt)

                x_sb = cpool.tile((C, NPAD), F32)
                y1e = cpool.tile((C + 1, NPAD), F32)
                cpax_sb = cpool.tile((P, NCONST_A), F32)
                cpb_sb = cpool.tile((P, NCONST_B), F32)

                nc.sync.dma_start(out=cpax_sb[:], in_=cpax[:])
                nc.scalar.dma_start(out=cpb_sb[:], in_=cpb[:])
                nc.sync.dma_start(out=x_sb[:, 128:384], in_=xall[:, 128:384])
                nc.sync.dma_start(out=y1e[C : C + 1, :], in_=onesr[:])
                nc.sync.dma_start(out=x_sb[:, 384:1408], in_=xall[:, 384:1408])
                nc.sync.dma_start(out=x_sb[:, 1408:NPAD], in_=xall[:, 1408:NPAD])

                gyc_sb = cpb_sb[:, OFF_GY : OFF_GY + S2]
                gxc_sb = cpb_sb[:, OFF_GX : OFF_GX + S2]
                nuv_sb = cpb_sb[:, OFF_NUV : OFF_NUV + 2 * NCHUNK]
                w1t_sb = cpax_sb[:C, OFF_W1 : OFF_W1 + C]
                w2b_sb = cpax_sb[: C + 1, OFF_W2B : OFF_W2B + C]
                sc1_sb = cpax_sb[:C, OFF_SC : OFF_SC + 1]
                bi1_sb = cpax_sb[:C, OFF_BI : OFF_BI + 1]
                x0_sb = cpax_sb[:C, OFF_X0 : OFF_X0 + P]

                # W scratch slabs: sq0/sq1/ss, wv holds W[n, slot*49+s]
                sq0 = cpool.tile((P, NCHUNK * S2), F32)
                sq1 = cpool.tile((P, NCHUNK * S2), F32)
                ss = cpool.tile((P, NCHUNK * S2), F32)
                wv = cpool.tile((P, NCHUNK * S2), F32)

                dma_out_engines = [nc.scalar, nc.sync]
                si = 0          # running store index for ring alternation

                for a, (a0, aw, slots) in enumerate(APIECES):
                    ps1 = ps_a.tile((C, NA), F32, tag="ps1")
                    rhs = x0_sb if a == 0 else x_sb[:, a0 : a0 + aw]
                    nc.tensor.matmul(ps1[:, :aw], lhsT=w1t_sb, rhs=rhs)
                    nc.scalar.activation(
                        y1e[:C, a0 : a0 + aw], ps1[:, :aw], AF.Relu,
                        bias=bi1_sb, scale=sc1_sb,
                    )

                    for k in slots:
                        nc.scalar.activation(
                            sq0[:, k * S2 : (k + 1) * S2], gyc_sb, AF.Square,
                            bias=nuv_sb[:, 2 * k : 2 * k + 1],
                        )
                        nc.scalar.activation(
                            sq1[:, k * S2 : (k + 1) * S2], gxc_sb, AF.Square,
                            bias=nuv_sb[:, 2 * k + 1 : 2 * k + 2],
                        )
                    psl = slice(slots[0] * S2, (slots[-1] + 1) * S2)
                    nc.vector.tensor_tensor(ss[:, psl], sq0[:, psl],
                                            sq1[:, psl], ALU.add)
                    nc.scalar.activation(sq0[:, psl], ss[:, psl], AF.Sqrt)
                    nc.scalar.activation(wv[:, psl], sq0[:, psl], AF.Relu,
                                         bias=1.0, scale=-1.0)

                    for k in slots:
                        # Slots 0-2 get low, disjoint, ascending scheduler
                        # priorities so their store pieces stream in order
                        # -- by default the Tile scheduler interleaves
                        # later slots' DVE multiplies between slot 0's
                        # pieces, stalling the first stores ~2us.
                        # (Readiness still gates scheduling, so this can't
                        # invert real dependencies.)
                        if k < 3:
                            saved_prio = tc.cur_priority
                            tc.cur_priority = 1 + 20 * k
                        try:
                            # F = [y1;1].T @ [w2t;b2] -> PSUM [n, c]
                            psf = ps_b.tile((P, C), F32, tag="psf")
                            nc.tensor.matmul(
                                psf[:], lhsT=y1e[:, k * P : (k + 1) * P],
                                rhs=w2b_sb,
                            )
                            o_sb = opool.tile((P, OUT_COLS), F32, tag="o")
                            wvk = wv[:, k * S2 : (k + 1) * S2]
                            c0 = 0
                            for cw in PIECES.get(k, [C]):
                                csl = slice(c0 * S2, (c0 + cw) * S2)
                                f_bc, w_bc = bass.broadcast_tensor_aps(
                                    psf[:, c0 : c0 + cw, None],
                                    wvk[:, None, :],
                                )
                                o_3d = o_sb[:, csl].rearrange(
                                    "p (c s) -> p c s", s=S2
                                )
                                nc.vector.tensor_tensor(o_3d, f_bc, w_bc,
                                                        ALU.mult)
                                for (p_lo, p_hi, r0) in SLOT_RUNS[k]:
                                    dma_out_engines[si % 2].dma_start(
                                        out=out[r0 : r0 + (p_hi - p_lo), csl],
                                        in_=o_sb[p_lo:p_hi, csl],
                                    )
                                    si += 1
                                c0 += cw
                        finally:
                            if k < 3:
                                tc.cur_priority = saved_prio
    nc.compile()
    return nc, names


_CACHE = {}


def _get_module():
    if "nc" not in _CACHE:
        _CACHE["nc"], _CACHE["names"] = _build_module()
    return _CACHE["nc"], _CACHE["names"]


def _prep_inputs(j2d_r, j2d_l, kp2d_o, feat_r, feat_l, feat_o,
                 w1, b1, bn_gamma, bn_beta, bn_mean, bn_var, w2, b2):
    """Host-side marshaling: shard batch, pack layouts. Returns in_maps."""
    f32 = np.float32
    # grid: grid[s] = (x[s%7], x[s//7]) with x = arange(7)+0.5
    x = (np.arange(S, dtype=f32) + 0.5)
    gy = np.tile(x, S) + EPS            # gy[s] = x[s%7] + eps
    gx = np.repeat(x, S) + EPS          # gx[s] = x[s//7] + eps

    scale = (bn_gamma.astype(f32) / np.sqrt(bn_var.astype(f32) + np.float32(1e-5)))
    bias1 = (b1.astype(f32) - bn_mean.astype(f32)) * scale + bn_beta.astype(f32)

    cpa0 = np.zeros((P, NCONST_A), f32)
    cpa0[:C, OFF_W1 : OFF_W1 + C] = w1.astype(f32).T
    cpa0[:C, OFF_W2B : OFF_W2B + C] = w2.astype(f32).T
    cpa0[C, OFF_W2B : OFF_W2B + C] = b2.astype(f32)
    cpa0[:C, OFF_SC] = scale
    cpa0[:C, OFF_BI] = bias1
    cpb0 = np.zeros((P, NCONST_B), f32)
    cpb0[:, OFF_GY : OFF_GY + S2] = gy
    cpb0[:, OFF_GX : OFF_GX + S2] = gx
    ones0 = np.ones((1, NPAD), f32)

    xcat = np.concatenate([feat_r, feat_l, feat_o], axis=2).astype(f32)  # (B,64,74)
    jcat = np.concatenate([j2d_r, j2d_l, kp2d_o], axis=1).astype(f32)   # (B,74,2)

    # device column s*128+p holds output row perm[s*128+p]; pad -> NLOC slot
    perm = np.empty(NPAD, np.int64)
    r = 0
    for s in range(NCHUNK):
        pads = PADS.get(s, set())
        for p in range(P):
            if p in pads:
                perm[s * P + p] = NLOC
            else:
                perm[s * P + p] = r
                r += 1
    assert r == NLOC

    in_maps = []
    for c in range(N_CORES):
        sl = slice(c * B_LOC, (c + 1) * B_LOC)
        xc = np.transpose(xcat[sl], (1, 0, 2)).reshape(C, NLOC)
        xpad = np.concatenate([xc, np.zeros((C, 1), f32)], axis=1)
        xa = np.ascontiguousarray(xpad[:, perm])
        cpac = cpa0.copy()
        cpac[:C, OFF_X0 : OFF_X0 + P] = xa[:, 0:P]
        # nuv[p, 2s+i] = -(uv[row(s,p), i] + 1) * 3.5; pad uv = 20 -> W = 0
        jc = np.full((NLOC + 1, 2), 20.0, f32)
        jc[:NLOC] = jcat[sl].reshape(NLOC, 2)
        nuv_flat = -(jc[perm] + np.float32(1.0)) * np.float32(3.5)  # (NPAD,2)
        cpbc = cpb0.copy()
        cpbc[:, OFF_NUV : OFF_NUV + 2 * NCHUNK] = (
            nuv_flat.reshape(NCHUNK, P, 2).transpose(1, 0, 2).reshape(P, 2 * NCHUNK)
        )
        in_maps.append(dict(xall=xa, cpax=cpac, cpb=cpbc, onesr=ones0))
    return in_maps


def kernel_with_results(trace=False, **inputs):
    nc, names = _get_module()
    in_maps_l = _prep_inputs(**inputs)
    in_maps = [{names[k]: v for k, v in m.items()} for m in in_maps_l]
    res = bass_utils.run_bass_kernel_spmd(
        nc, in_maps, core_ids=list(range(N_CORES)), trace=trace
    )
    out_name = names["out"]
    parts = [
        res.results[c][out_name].reshape(B_LOC, J * C, S, S) for c in range(N_CORES)
    ]
    full = np.concatenate(parts, axis=0)
    return full, res


def kernel(**inputs):
    full, _ = kernel_with_results(trace=False, **inputs)
    return full


# revision 17
# speedup vs baseline: 1.0882x; 1.0882x over previous
"""Trainium2 Bass kernel for nn_Enhancement_11819749999257.

Computes: 3x (1x1-conv MLP w/ BN+relu) feature embeddings + soft scatter of
per-joint features onto a 7x7 grid ("bone projection"), concatenated.

Full output: (256, 4736, 7, 7) f32 = 237 MB  -> memory(write)-bound.
Per core: 29.7 MB of stores at the ~360 GB/s HBM wall ~= 82 us; the craft
is all in the ramp (first store ASAP), keeping the SDMA engines gap-free,
and the tail (every engine finishing together).

Strategy (pure data parallel over batch, 8 cores x 32 batch items):
  - n = b_local*74 + j  flattens (batch item, joint). Per-core output
    (32, 4736, 49) is contiguous as rows n: out[n, c*49+s]. Rows are
    processed in 19 slots of 128 partitions. NLOC = 2368 = 19*128 - 64;
    the 64 pad rows are placed where they help the DMA tail:
      * 52 pads on partitions 0-51 of slot 9 (mid-stream),
      * 12 pads on SDMA-engine-15's partitions {124-127 of slot 17,
        92-95 & 124-127 of slot 18}: engine 15 drains ~6% slower than
        the others (known TRN2 trait), so skipping its descriptors in
        the final slots lets all 16 engines finish together.
  - Each slot is ONE store (128 x 12544 B descriptors) issued as soon as
    the slot's DVE multiply finishes; slots 0-2 are split into 4/2/2
    column pieces so the first store issues early. DVE production
    (~3.5us/slot) outruns DMA consumption (~4.2us/slot), so after slot 0
    the two HWDGE rings stream back-to-back stores to the end.
  - MLP: w1/w2 are 64x64; BN (eval) folded into per-channel scale/bias on
    host. PE matmuls: y1 = relu(scale*(w1 @ x) + bias) in column pieces;
    per slot F = [y1;1].T @ [w2.T; b2] (K=65 folds the b2 add) lands in
    PSUM in [n-partition, c-free] layout. The first 128 x-columns ride
    inside the cpa constants DMA so the first matmul waits on a single
    load; the ones row of y1e comes from a tiny DMA (input `onesr`).
    GpSimd (slow Q7 start) is never used.
  - Grid weights W[n, s] = relu(1 - sqrt((gy_s+eps-u_n)^2 + (gx_s+eps-v_n)^2))
    via ACT Square (per-partition bias = -uv), one DVE add per group,
    ACT Sqrt, ACT Relu. The compiler's two ACT table loads (set 0 at
    block entry + the sqrt set) run 6.6-9.2us, overlapping the input
    DMAs; cpb lands ~9.1us so the W chain is not table-bound.
  - Scatter: OUT[n, c*49+s] = F[n, c] * W[n, s] -- one DVE tensor_tensor
    mult per slot with stride-0 broadcast APs.
  - Stores alternate between the two HWDGE rings (scalar / sync); all
    input DMAs except cpb ride sync so the ACT sequencer stays free.
"""

import numpy as np

import concourse.bass as bass
import concourse.mybir as mybir
from concourse import bacc, bass_utils
from concourse.tile import TileContext

F32 = mybir.dt.float32
AF = mybir.ActivationFunctionType
ALU = mybir.AluOpType

N_CORES = 8
B = 256
B_LOC = B // N_CORES      # 32
J = 74                    # 21 + 21 + 32 joints, concat order r, l, o
C = 64
S = 7
S2 = S * S                # 49
NLOC = B_LOC * J          # 2368 rows per core
P = 128
NCHUNK = 19               # slots
NPAD = NCHUNK * P         # 2432
OUT_COLS = C * S2         # 3136
EPS = 1.0e-6
NA = 512                  # max phase-A column piece
# phase-A pieces (col0, width, slots); piece 0 is one slot and rides the
# cpa DMA so the first matmul starts as soon as possible
APIECES = [
    (0, 128, [0]),
    (128, 256, [1, 2]),
    (384, 512, [3, 4, 5, 6]),
    (896, 512, [7, 8, 9, 10]),
    (1408, 512, [11, 12, 13, 14]),
    (1920, 512, [15, 16, 17, 18]),
]
# W-chain batching groups == the APIECES slot lists
# store channel-piece widths per slot: slot 0 finely split for an early
# first store; slots 1-6 split 2x so early stores chase the DVE
# multiplies piece-wise (DVE production only outruns DMA consumption by
# ~15%, so half-slot granularity avoids whole-slot wait bubbles until
# DVE builds a lead)
PIECES = {0: [8, 16, 16, 24], 1: [16, 16, 32], 2: [16, 16, 32],
          3: [32, 32], 4: [32, 32], 5: [32, 32], 6: [32, 32]}

# pad partitions per slot (64 total): 48 mid-stream on slot 9 (its
# 80-partition store sprays all 16 SDMA engines: stores assign
# descriptors to engines in 4-partition slices from engine 0, wrapping
# to all 16 only when the count is divisible by 16), 16 in the last
# slot so its two stores (60 and 52 partitions -> engines 0-14 and
# 0-12) skip exactly the ~7%-slow engine 15, letting its queue drain
# in time with the others' final slot.
PADS = {9: set(range(0, 48)), 18: set(range(112, 128))}
# contiguous valid-partition runs per slot
RUNS = {9: [(48, 128)], 18: [(0, 60), (60, 112)]}

# packed-constants column layout:
# cpax = [w1t|w2b|sc1|bi1|x0] (MLP path, sync ring), cpb = [gyc|gxc|nuv]
# (grid path, scalar ring); onesr = the K=65 ones row (sync ring).
OFF_W1 = 0
OFF_W2B = OFF_W1 + C            # 64
OFF_SC = OFF_W2B + C            # 128
OFF_BI = OFF_SC + 1             # 129
OFF_X0 = OFF_BI + 1             # 130
NCONST_A = OFF_X0 + P           # 258
OFF_GY = 0
OFF_GX = OFF_GY + S2            # 49
OFF_NUV = OFF_GX + S2           # 98
NCONST_B = OFF_NUV + 2 * NCHUNK  # 136


def _slot_layout():
    """Per slot: (dram row0, [(p_lo, p_hi, row0_of_run), ...])."""
    out = []
    r = 0
    for s in range(NCHUNK):
        runs = []
        for (p_lo, p_hi) in RUNS.get(s, [(0, P)]):
            runs.append((p_lo, p_hi, r))
            r += p_hi - p_lo
        out.append(runs)
    assert r == NLOC
    return out


SLOT_RUNS = _slot_layout()


def _build_module():
    nc = bacc.Bacc(None)
    names = {}
    with TileContext(nc) as tc:
        with tc.tile_pool(name="dram", bufs=1, space="DRAM") as dram:
            xall = dram.tile((C, NPAD), F32, kind="ExternalInput", name="xall")
            cpax = dram.tile((P, NCONST_A), F32, kind="ExternalInput", name="cpax")
            cpb = dram.tile((P, NCONST_B), F32, kind="ExternalInput", name="cpb")
            onesr = dram.tile((1, NPAD), F32, kind="ExternalInput", name="onesr")
            out = dram.tile((NLOC, OUT_COLS), F32, kind="ExternalOutput", name="out")
            for key, ap in (("xall", xall), ("cpax", cpax), ("cpb", cpb),
                            ("onesr", onesr), ("out", out)):
                names[key] = ap.tensor.name

            with (
                tc.tile_pool(name="consts", bufs=1) as cpool,
                tc.tile_pool(name="ps_a", bufs=2, space="PSUM") as ps_a,
                tc.tile_pool(name="ps_b", bufs=4, space="PSUM") as ps_b,
                tc.tile_pool(name="outs", bufs=6) as opool,
            ):
                # Warm the ACT sqrt-set table (also holds Square/Relu): the
                # compiler inserts the set-3 table load right before the
                # first Sqrt in ACT program order, so a dependency-light
                # dummy Sqrt up front hoists the ~1.3us load to t~6.6-9.2,
                # overlapping the input DMAs instead of the W chain.
                scr = cpool.tile((1, 8), F32)
                scro = cpool.tile((1, 8), F32)
                nc.vector.memset(scr[:], 0.0625)
                nc.scalar.activation(scro[:, 0:2], scr[:, 0:2], AF.Sqrt)

                x_sb = cpool.tile((C, NPAD), F32)
                y1e = cpool.tile((C + 1, NPAD), F32)
                cpax_sb = cpool.tile((P, NCONST_A), F32)
                cpb_sb = cpool.tile((P, NCONST_B), F32)

                nc.sync.dma_start(out=cpax_sb[:], in_=cpax[:])
                nc.scalar.dma_start(out=cpb_sb[:], in_=cpb[:])
                nc.sync.dma_start(out=x_sb[:, 128:384], in_=xall[:, 128:384])
                nc.sync.dma_start(out=y1e[C : C + 1, :], in_=onesr[:])
                nc.sync.dma_start(out=x_sb[:, 384:1408], in_=xall[:, 384:1408])
                nc.sync.dma_start(out=x_sb[:, 1408:NPAD], in_=xall[:, 1408:NPAD])

                gyc_sb = cpb_sb[:, OFF_GY : OFF_GY + S2]
                gxc_sb = cpb_sb[:, OFF_GX : OFF_GX + S2]
                nuv_sb = cpb_sb[:, OFF_NUV : OFF_NUV + 2 * NCHUNK]
                w1t_sb = cpax_sb[:C, OFF_W1 : OFF_W1 + C]
                w2b_sb = cpax_sb[: C + 1, OFF_W2B : OFF_W2B + C]
                sc1_sb = cpax_sb[:C, OFF_SC : OFF_SC + 1]
                bi1_sb = cpax_sb[:C, OFF_BI : OFF_BI + 1]
                x0_sb = cpax_sb[:C, OFF_X0 : OFF_X0 + P]

                # W scratch slabs: sq0/sq1/ss, wv holds W[n, slot*49+s]
                sq0 = cpool.tile((P, NCHUNK * S2), F32)
                sq1 = cpool.tile((P, NCHUNK * S2), F32)
                ss = cpool.tile((P, NCHUNK * S2), F32)
                wv = cpool.tile((P, NCHUNK * S2), F32)

                dma_out_engines = [nc.scalar, nc.sync]
                si = 0          # running store index for ring alternation

                for a, (a0, aw, slots) in enumerate(APIECES):
                    ps1 = ps_a.tile((C, NA), F32, tag="ps1")
                    rhs = x0_sb if a == 0 else x_sb[:, a0 : a0 + aw]
                    nc.tensor.matmul(ps1[:, :aw], lhsT=w1t_sb, rhs=rhs)
                    nc.scalar.activation(
                        y1e[:C, a0 : a0 + aw], ps1[:, :aw], AF.Relu,
                        bias=bi1_sb, scale=sc1_sb,
                    )

                    for k in slots:
                        nc.scalar.activation(
                            sq0[:, k * S2 : (k + 1) * S2], gyc_sb, AF.Square,
                            bias=nuv_sb[:, 2 * k : 2 * k + 1],
                        )
                        nc.scalar.activation(
                            sq1[:, k * S2 : (k + 1) * S2], gxc_sb, AF.Square,
                            bias=nuv_sb[:, 2 * k + 1 : 2 * k + 2],
                        )
                    psl = slice(slots[0] * S2, (slots[-1] + 1) * S2)
                    nc.vector.tensor_tensor(ss[:, psl], sq0[:, psl],
                                            sq1[:, psl], ALU.add)
                    nc.scalar.activation(sq0[:, psl], ss[:, psl], AF.Sqrt)
                    nc.scalar.activation(wv[:, psl], sq0[:, psl], AF.Relu,
                                         bias=1.0, scale=-1.0)

                    for k in slots:
                        # F = [y1;1].T @ [w2t;b2] -> PSUM [128 (n), 64 (c)]
                        psf = ps_b.tile((P, C), F32, tag="psf")
                        nc.tensor.matmul(
                            psf[:], lhsT=y1e[:, k * P : (k + 1) * P],
                            rhs=w2b_sb,
                        )
                        o_sb = opool.tile((P, OUT_COLS), F32, tag="o")
                        wvk = wv[:, k * S2 : (k + 1) * S2]
                        c0 = 0
                        for cw in PIECES.get(k, [C]):
                            csl = slice(c0 * S2, (c0 + cw) * S2)
                            f_bc, w_bc = bass.broadcast_tensor_aps(
                                psf[:, c0 : c0 + cw, None],
                                wvk[:, None, :],
                            )
                            o_3d = o_sb[:, csl].rearrange(
                                "p (c s) -> p c s", s=S2
                            )
                            nc.vector.tensor_tensor(o_3d, f_bc, w_bc,
                                                    ALU.mult)
                            for (p_lo, p_hi, r0) in SLOT_RUNS[k]:
                                dma_out_engines[si % 2].dma_start(
                                    out=out[r0 : r0 + (p_hi - p_lo), csl],
                                    in_=o_sb[p_lo:p_hi, csl],
                                )
                                si += 1
                            c0 += cw
    nc.compile()
    return nc, names


_CACHE = {}


def _get_module():
    if "nc" not in _CACHE:
        _CACHE["nc"], _CACHE["names"] = _build_module()
    return _CACHE["nc"], _CACHE["names"]


def _prep_inputs(j2d_r, j2d_l, kp2d_o, feat_r, feat_l, feat_o,
                 w1, b1, bn_gamma, bn_beta, bn_mean, bn_var, w2, b2):
    """Host-side marshaling: shard batch, pack layouts. Returns in_maps."""
    f32 = np.float32
    # grid: grid[s] = (x[s%7], x[s//7]) with x = arange(7)+0.5
    x = (np.arange(S, dtype=f32) + 0.5)
    gy = np.tile(x, S) + EPS            # gy[s] = x[s%7] + eps
    gx = np.repeat(x, S) + EPS          # gx[s] = x[s//7] + eps

    scale = (bn_gamma.astype(f32) / np.sqrt(bn_var.astype(f32) + np.float32(1e-5)))
    bias1 = (b1.astype(f32) - bn_mean.astype(f32)) * scale + bn_beta.astype(f32)

    cpa0 = np.zeros((P, NCONST_A), f32)
    cpa0[:C, OFF_W1 : OFF_W1 + C] = w1.astype(f32).T
    cpa0[:C, OFF_W2B : OFF_W2B + C] = w2.astype(f32).T
    cpa0[C, OFF_W2B : OFF_W2B + C] = b2.astype(f32)
    cpa0[:C, OFF_SC] = scale
    cpa0[:C, OFF_BI] = bias1
    cpb0 = np.zeros((P, NCONST_B), f32)
    cpb0[:, OFF_GY : OFF_GY + S2] = gy
    cpb0[:, OFF_GX : OFF_GX + S2] = gx
    ones0 = np.ones((1, NPAD), f32)

    xcat = np.concatenate([feat_r, feat_l, feat_o], axis=2).astype(f32)  # (B,64,74)
    jcat = np.concatenate([j2d_r, j2d_l, kp2d_o], axis=1).astype(f32)   # (B,74,2)

    # device column s*128+p holds output row perm[s*128+p]; pad -> NLOC slot
    perm = np.empty(NPAD, np.int64)
    r = 0
    for s in range(NCHUNK):
        pads = PADS.get(s, set())
        for p in range(P):
            if p in pads:
                perm[s * P + p] = NLOC
            else:
                perm[s * P + p] = r
                r += 1
    assert r == NLOC

    in_maps = []
    for c in range(N_CORES):
        sl = slice(c * B_LOC, (c + 1) * B_LOC)
        xc = np.transpose(xcat[sl], (1, 0, 2)).reshape(C, NLOC)
        xpad = np.concatenate([xc, np.zeros((C, 1), f32)], axis=1)
        xa = np.ascontiguousarray(xpad[:, perm])
        cpac = cpa0.copy()
        cpac[:C, OFF_X0 : OFF_X0 + P] = xa[:, 0:P]
        # nuv[p, 2s+i] = -(uv[row(s,p), i] + 1) * 3.5; pad uv = 20 -> W = 0
        jc = np.full((NLOC + 1, 2), 20.0, f32)
        jc[:NLOC] = jcat[sl].reshape(NLOC, 2)
        nuv_flat = -(jc[perm] + np.float32(1.0)) * np.float32(3.5)  # (NPAD,2)
        cpbc = cpb0.copy()
        cpbc[:, OFF_NUV : OFF_NUV + 2 * NCHUNK] = (
            nuv_flat.reshape(NCHUNK, P, 2).transpose(1, 0, 2).reshape(P, 2 * NCHUNK)
        )
        in_maps.append(dict(xall=xa, cpax=cpac, cpb=cpbc, onesr=ones0))
    return in_maps


def kernel_with_results(trace=False, **inputs):
    nc, names = _get_module()
    in_maps_l = _prep_inputs(**inputs)
    in_maps = [{names[k]: v for k, v in m.items()} for m in in_maps_l]
    res = bass_utils.run_bass_kernel_spmd(
        nc, in_maps, core_ids=list(range(N_CORES)), trace=trace
    )
    out_name = names["out"]
    parts = [
        res.results[c][out_name].reshape(B_LOC, J * C, S, S) for c in range(N_CORES)
    ]
    full = np.concatenate(parts, axis=0)
    return full, res


def kernel(**inputs):
    full, _ = kernel_with_results(trace=False, **inputs)
    return full
